# revision 1
# baseline (speedup 1.0000x reference)
"""Trainium2 Bass kernel for nn_ConditionPooler (ragged cross-attention pooler).

Algorithm (per core, data-parallel over B=16 scenes, 2 scenes/core on 8 cores):
  scores^T[n,(h,t)] = feat @ A^T      where A[(h,t),c] = sum_d qh[t,h,d] w_k[h*DH+d,c]
  P = exp(scores)  (no max-subtraction needed; scores ~ N(0,1); b_k_in cancels in
                    softmax since it shifts scores by a constant per (h,t))
  U[(h,t),c] = sum_n P[n,(h,t)] feat[n,c]   (per scene; padded rows contribute 0)
  den[(h,t)] = sum_n P[n,(h,t)] mask[n]
  Uhat = U / den; attn_h = Uhat_h @ w_v_h^T; out = attn @ w_o^T + b_o (+ w_o b_v_in)
  h = out + query; z = (h-mu)/std; ff = gelu(z @ (w1*g)^T + b1eff) @ w2^T + b2
  result = h + ff
Scenes are padded to a common length P (multiple of 128) so the SPMD program is
static; segment boundaries are computed on the host from batch_idx.
"""

import numpy as np

C = 512
T = 32
H = 8
DH = C // H
NCORES = 8

_CACHE = {}


def _apply_tile_patch():
    """This walrus build allows only one sem wait on CTRL-encoded (Drain)
    instructions; TileContext's tail drain carries the whole global clock.
    Split the extra waits onto standalone sync-engine nops."""
    import concourse.tile as tile_mod
    import concourse.mybir as mybir
    from concourse.vector_clock import ScopedClock

    if getattr(tile_mod.TileContext, "_drain_patched", False):
        return

    def _patched(self, tick_clock, wait_clock):
        nc = self.nc
        drain_inst = nc.sync.drain()
        wait_clock.add_sem_waits(
            drain_inst.ins, ScopedClock({None: tick_clock.global_clock})
        )
        si = drain_inst.ins.sync_info
        if si is not None and si.on_wait is not None and len(si.on_wait) > 1:
            waits = list(si.on_wait)
            si.on_wait = waits[:1]
            for w in waits[1:]:
                nop = nc.sync.nop(nofuse=True)
                nsi = nop.ins.sync_info
                if nsi is None:
                    nop.ins.sync_info = mybir.SyncInfo(on_wait=[w], on_update=[])
                else:
                    nsi.on_wait = [w]
        nc.all_engine_barrier()
        assert self.sems is not None
        popped = nc._tile_sem_poison_stack.pop()
        assert popped is self._sem_poison
        nc.clear_and_free_semaphores(list(self.sems.allocated().values()))
        nc.all_engine_barrier()

    tile_mod.TileContext._drain_and_barrier = _patched
    tile_mod.TileContext._drain_patched = True


def _split_multi_waits(nc):
    """This walrus build caps sync waits at 1 per instruction (2 for
    EventSemaphore). Tile emits several on some instructions; hoist the
    extras onto same-engine NoOps inserted just before."""
    import concourse.mybir as mybir

    cnt = [0]
    for f in nc.m.functions:
        for b in f.blocks:
            newlist = []
            for inst in b.instructions:
                si = inst.sync_info
                if si is not None and si.on_wait is not None and len(si.on_wait) > 1:
                    waits = list(si.on_wait)
                    for w in waits[:-1]:
                        cnt[0] += 1
                        nop = mybir.InstNoOp(
                            name=f"I-wsplit-{cnt[0]}", ins=[], outs=[]
                        )
                        nop.engine = inst.engine
                        nop.sync_info = mybir.SyncInfo(on_wait=[w], on_update=[])
                        newlist.append(nop)
                    si.on_wait = waits[-1:]
                newlist.append(inst)
            b.instructions = newlist


def _build(P, S, compute="f32r", chunk=4, split=True):
    """Build the per-core SPMD Bass program. P = padded scene length
    (multiple of 128), S = scenes per core, compute in {"f32r","f32","bf16"}."""
    import concourse.bass as bass
    import concourse.mybir as mybir
    import concourse.tile as tile

    _apply_tile_patch()

    f32 = mybir.dt.float32
    bf16 = mybir.dt.bfloat16
    f32r = mybir.dt.float32r
    HT = H * T  # 256
    NT = P // 128
    assert P % 128 == 0
    AF = mybir.ActivationFunctionType
    ALU = mybir.AluOpType

    feat_dt = bf16 if compute == "bf16" else f32

    def mm(ap):  # matmul-operand dtype view for the streaming phase
        if compute == "f32r":
            return ap.bitcast(f32r)
        return ap

    nc = bass.Bass()
    featp = nc.dram_tensor("featp", [S * P, C], feat_dt, kind="ExternalInput")
    maskp = nc.dram_tensor("maskp", [128, S * NT], feat_dt, kind="ExternalInput")
    akT_d = nc.dram_tensor("akT", [C, HT], feat_dt, kind="ExternalInput")
    wvT_d = nc.dram_tensor("wvT", [C, C], bf16, kind="ExternalInput")
    woT_d = nc.dram_tensor("woT", [C, C], bf16, kind="ExternalInput")
    w1gT_d = nc.dram_tensor("w1gT", [C, 2 * C], bf16, kind="ExternalInput")
    b1e_d = nc.dram_tensor("b1e", [1, 2 * C], bf16, kind="ExternalInput")
    w2T_d = nc.dram_tensor("w2T", [2 * C, C], bf16, kind="ExternalInput")
    b2e_d = nc.dram_tensor("b2e", [1, C], bf16, kind="ExternalInput")
    qb_d = nc.dram_tensor("qb", [T, C], f32, kind="ExternalInput")
    id_d = nc.dram_tensor("ident", [128, 128], feat_dt, kind="ExternalInput")
    idb_d = nc.dram_tensor("identb", [128, 128], bf16, kind="ExternalInput")
    outp = nc.dram_tensor("outp", [S * T, C], f32, kind="ExternalOutput")

    NCH = NT // chunk  # chunks per scene
    assert NT % chunk == 0

    with tile.TileContext(nc) as tc:
        with tc.tile_pool(name="const", bufs=1) as const:
            # constants
            akT = const.tile([128, 4, HT], feat_dt, tag="akT")
            nc.sync.dma_start(akT[:], akT_d.rearrange("(j p) f -> p j f", p=128))
            wvT = const.tile([128, 4, C], bf16, tag="wvT")
            woT = const.tile([128, 4, C], bf16, tag="woT")
            w1gT = const.tile([128, 4, 2 * C], bf16, tag="w1gT")
            w2T = const.tile([128, 8, C], bf16, tag="w2T")
            b1e = const.tile([1, 2 * C], bf16, tag="b1e")
            b2e = const.tile([1, C], bf16, tag="b2e")
            qb2 = const.tile([2 * T, C], f32, tag="qb2")

            def load_epi_weights():
                # SWDGE so these 3.3MB don't head-of-line block the feat
                # stream on HWDGE; issued mid-stream, needed only at the end.
                g = nc.gpsimd
                g.dma_start(wvT[:], wvT_d.rearrange("(j p) f -> p j f", p=128))
                g.dma_start(woT[:], woT_d.rearrange("(j p) f -> p j f", p=128))
                g.dma_start(w1gT[:], w1gT_d.rearrange("(j p) f -> p j f", p=128))
                g.dma_start(w2T[:], w2T_d.rearrange("(j p) f -> p j f", p=128))
                g.dma_start(b1e[:], b1e_d[:])
                g.dma_start(b2e[:], b2e_d[:])
                for s in range(S):
                    g.dma_start(qb2[s * T : (s + 1) * T, :], qb_d[:])
            ident = const.tile([128, 128], feat_dt, tag="ident")
            nc.sync.dma_start(ident[:], id_d[:])
            identb = const.tile([128, 128], bf16, tag="identb")
            nc.sync.dma_start(identb[:], idb_d[:])
            masks = const.tile([128, S * NT], feat_dt, tag="masks")
            nc.sync.dma_start(masks[:], maskp[:])
            ones = const.tile([1, 2 * T], bf16, tag="ones")
            nc.vector.memset(ones[:], 1.0)

            featv = featp.rearrange("(s g i p) c -> s g p i c", p=128, i=chunk, g=NCH)

            with tc.tile_pool(name="epiA", bufs=1) as epiA:
                rden = epiA.tile([128, 2, S], f32, tag="rden")
                Uhat = [
                    epiA.tile([128, 2, C], bf16, tag=f"Uh{s}", name=f"Uh{s}")
                    for s in range(S)
                ]
                dacc = epiA.tile([128, 2, S], f32, tag="dacc")
                with (
                    tc.tile_pool(name="psU", bufs=1, space="PSUM") as psU_pool,
                    tc.tile_pool(name="fb", bufs=3) as fpool,
                    tc.tile_pool(name="sb", bufs=4) as spool,
                    tc.tile_pool(name="pstr", bufs=2, space="PSUM") as pstr,
                    tc.tile_pool(name="pssc", bufs=2, space="PSUM") as pssc,
                    tc.tile_pool(name="psd", bufs=2, space="PSUM") as psd,
                ):
                    def emit_U(st):
                        PTp, Fp, sp, ip, Upsp = st
                        ps_d = psd.tile([128, 2], f32, tag="den", name="ps_d")
                        for h2 in range(2):
                            nc.tensor.matmul(
                                Upsp[h2][:],
                                mm(PTp[:, h2 * 128 : (h2 + 1) * 128]),
                                mm(Fp),
                                start=(ip == 0),
                                stop=(ip == NT - 1),
                            )
                            nc.tensor.matmul(
                                ps_d[:, h2 : h2 + 1],
                                mm(PTp[:, h2 * 128 : (h2 + 1) * 128]),
                                mm(masks[:, sp * NT + ip : sp * NT + ip + 1]),
                                start=True,
                                stop=True,
                            )
                        if ip == 0:
                            nc.vector.tensor_copy(dacc[:, :, sp], ps_d[:])
                        else:
                            nc.vector.tensor_add(
                                dacc[:, :, sp], dacc[:, :, sp], ps_d[:]
                            )

                    def finish_scene(st):
                        emit_U(st)
                        sp, Upsp = st[2], st[4]
                        nc.vector.reciprocal(rden[:, :, sp], dacc[:, :, sp])
                        for h2 in range(2):
                            nc.scalar.activation(
                                Uhat[sp][:, h2, :],
                                Upsp[h2][:],
                                AF.Copy,
                                scale=rden[:, h2, sp : sp + 1],
                            )

                    pend = None
                    for s in range(S):
                        Ups = [
                            psU_pool.tile(
                                [128, C], f32, tag=f"U{h2}", name=f"U{s}{h2}"
                            )
                            for h2 in range(2)
                        ]
                        for g in range(NCH):
                            Fc = fpool.tile([128, chunk, C], feat_dt, tag="F")
                            nc.sync.dma_start(Fc[:], featv[s, g])
                            if s == 0 and g == 0:
                                load_epi_weights()
                            for ii in range(chunk):
                                i = g * chunk + ii
                                F = Fc[:, ii, :]
                                ps_tr = pstr.tile([128, C], feat_dt, tag="tr")
                                for j in range(4):
                                    nc.tensor.transpose(
                                        ps_tr[:, j * 128 : (j + 1) * 128],
                                        F[:, j * 128 : (j + 1) * 128],
                                        ident[:],
                                    )
                                FT = spool.tile([128, C], feat_dt, tag="FT")
                                nc.vector.tensor_copy(FT[:], ps_tr[:])
                                ps_s = pssc.tile([128, HT], f32, tag="sc")
                                for j in range(4):
                                    nc.tensor.matmul(
                                        ps_s[:],
                                        mm(FT[:, j * 128 : (j + 1) * 128]),
                                        mm(akT[:, j, :]),
                                        start=(j == 0),
                                        stop=(j == 3),
                                    )
                                PT = spool.tile([128, HT], feat_dt, tag="PT")
                                nc.scalar.activation(PT[:], ps_s[:], AF.Exp)
                                if pend is not None:
                                    if pend[2] != s:
                                        finish_scene(pend)
                                    else:
                                        emit_U(pend)
                                pend = (PT, F, s, i, Ups)
                    finish_scene(pend)
                # ---- epilogue (PSUM banks from streaming now free) ----
                with (
                    tc.tile_pool(name="epiB", bufs=1) as epi,
                    tc.tile_pool(name="pse", bufs=2, space="PSUM") as pse,
                    tc.tile_pool(name="pacc", bufs=1, space="PSUM") as pacc,
                ):
                    # transpose Uhat -> UT [c' (4x128), (s,ht)]
                    UT = epi.tile([128, 4, S, HT], bf16, tag="UT")
                    for s in range(S):
                        ps_u = pse.tile([128, 4, HT], bf16, tag="tre", name="ps_u")
                        for h2 in range(2):
                            for jc in range(4):
                                nc.tensor.transpose(
                                    ps_u[:, jc, h2 * 128 : (h2 + 1) * 128],
                                    Uhat[s][:, h2, jc * 128 : (jc + 1) * 128],
                                    identb[:],
                                )
                        nc.any.tensor_copy(UT[:, :, s, :], ps_u[:])

                    # attention value projection: attnT[(hd), (s,t)]
                    at_ps = pacc.tile([128, 4, S, T], f32, tag="at")
                    for gq in range(4):
                        for hh in range(2):
                            h = 2 * gq + hh
                            for jc in range(4):
                                nc.tensor.matmul(
                                    at_ps[hh * 64 : (hh + 1) * 64, gq, :, :],
                                    wvT[:, jc, h * DH : (h + 1) * DH].bitcast(bf16),
                                    UT[:, jc, :, h * T : (h + 1) * T],
                                    start=(jc == 0),
                                    stop=(jc == 3),
                                )
                    at_sb = epi.tile([128, 4, S, T], bf16, tag="at_sb")
                    nc.any.tensor_copy(at_sb[:], at_ps[:])

                    # output projection -> h = out + query + b
                    ph = pacc.tile([S * T, C], f32, tag="ph")
                    for gq in range(4):
                        nc.tensor.matmul(
                            ph[:],
                            at_sb[:, gq, :, :],
                            woT[:, gq, :],
                            start=(gq == 0),
                            stop=(gq == 3),
                        )
                    h_sb = epi.tile([S * T, C], f32, tag="h")
                    nc.vector.tensor_add(h_sb[:], ph[:], qb2[:])

                    # layernorm -> z (bf16)
                    ssum = epi.tile([S * T, 1], f32, tag="ssum")
                    nc.vector.reduce_sum(ssum[:], h_sb[:], axis=mybir.AxisListType.X)
                    mu = epi.tile([S * T, 1], f32, tag="mu")
                    nc.scalar.mul(mu[:], ssum[:], 1.0 / C)
                    cen = epi.tile([S * T, C], f32, tag="cen")
                    nc.vector.tensor_scalar_sub(cen[:], h_sb[:], mu[:])
                    sq = epi.tile([S * T, C], f32, tag="sq")
                    ssq = epi.tile([S * T, 1], f32, tag="ssq")
                    nc.scalar.activation(sq[:], cen[:], AF.Square, accum_out=ssq[:])
                    epsc = epi.tile([S * T, 1], f32, tag="epsc")
                    nc.vector.memset(epsc[:], 1e-5)
                    std = epi.tile([S * T, 1], f32, tag="std")
                    nc.scalar.activation(
                        std[:], ssq[:], AF.Sqrt, bias=epsc[:], scale=1.0 / C
                    )
                    rstd = epi.tile([S * T, 1], f32, tag="rstd")
                    nc.vector.reciprocal(rstd[:], std[:])
                    z = epi.tile([S * T, C], bf16, tag="z")
                    nc.vector.tensor_scalar_mul(z[:], cen[:], rstd[:])

                    # zT
                    zT = epi.tile([128, 4, S * T], bf16, tag="zT")
                    ps_z = pse.tile([128, 4, S * T], bf16, tag="tre", name="ps_z")
                    for jc in range(4):
                        nc.tensor.transpose(
                            ps_z[:, jc, :],
                            z[:, jc * 128 : (jc + 1) * 128],
                            identb[: S * T, : S * T],
                        )
                    nc.any.tensor_copy(zT[:], ps_z[:])

                    # ff1 + gelu
                    gm = epi.tile([S * T, 2, C], bf16, tag="gm")
                    for half in range(2):
                        pf = pacc.tile([S * T, C], f32, tag=f"pf{half}")
                        for jc in range(4):
                            nc.tensor.matmul(
                                pf[:],
                                zT[:, jc, :],
                                w1gT[:, jc, half * C : (half + 1) * C],
                                start=(jc == 0),
                                stop=False,
                            )
                        nc.tensor.matmul(
                            pf[:],
                            ones[:],
                            b1e[:, half * C : (half + 1) * C],
                            start=False,
                            stop=True,
                        )
                        nc.scalar.activation(gm[:, half, :], pf[:], AF.Gelu)

                    # gmT
                    gmT = epi.tile([128, 8, S * T], bf16, tag="gmT")
                    for half in range(2):
                        ps_g = pse.tile([128, 4, S * T], bf16, tag="tre", name="ps_g")
                        for jc in range(4):
                            nc.tensor.transpose(
                                ps_g[:, jc, :],
                                gm[:, half, jc * 128 : (jc + 1) * 128],
                                identb[: S * T, : S * T],
                            )
                        nc.any.tensor_copy(gmT[:, half * 4 : (half + 1) * 4, :], ps_g[:])

                    # ff2 + residual
                    po = pacc.tile([S * T, C], f32, tag="po")
                    for k in range(8):
                        nc.tensor.matmul(
                            po[:], gmT[:, k, :], w2T[:, k, :], start=(k == 0), stop=False
                        )
                    nc.tensor.matmul(po[:], ones[:], b2e[:], start=False, stop=True)
                    fin = epi.tile([S * T, C], f32, tag="fin")
                    nc.vector.tensor_add(fin[:], h_sb[:], po[:])
                    nc.sync.dma_start(outp[:], fin[:])

    if split:
        _split_multi_waits(nc)
    return nc


def _pick_chunk(NT):
    for c in (6, 5, 4, 3, 2, 1):
        if NT % c == 0:
            return c
    return 1


def _host_prep(inputs, compute):
    import ml_dtypes

    feat = np.asarray(inputs["feat"], dtype=np.float32)
    batch_idx = np.asarray(inputs["batch_idx"]).astype(np.int64)
    B = int(np.asarray(inputs["batch_size"]))
    query = np.asarray(inputs["query"], dtype=np.float32)
    g_q = np.asarray(inputs["g_q"], np.float32)
    b_q = np.asarray(inputs["b_q"], np.float32)
    w_q = np.asarray(inputs["w_q"], np.float32)
    w_k = np.asarray(inputs["w_k"], np.float32)
    w_v = np.asarray(inputs["w_v"], np.float32)
    b_q_in = np.asarray(inputs["b_q_in"], np.float32)
    b_v_in = np.asarray(inputs["b_v_in"], np.float32)
    w_o = np.asarray(inputs["w_o"], np.float32)
    b_o = np.asarray(inputs["b_o"], np.float32)
    g_ff = np.asarray(inputs["g_ff"], np.float32)
    b_ff = np.asarray(inputs["b_ff"], np.float32)
    w1 = np.asarray(inputs["w1"], np.float32)
    b1 = np.asarray(inputs["b1"], np.float32)
    w2 = np.asarray(inputs["w2"], np.float32)
    b2 = np.asarray(inputs["b2"], np.float32)

    N = feat.shape[0]
    S = B // NCORES
    counts = np.bincount(batch_idx, minlength=B)
    offs = np.concatenate([[0], np.cumsum(counts)])
    NT = max(1, int(np.ceil(counts.max() / 128)))
    while _pick_chunk(NT) < 3 and NT > 2:
        NT += 1
    P = NT * 128

    bf = ml_dtypes.bfloat16
    feat_np_dt = bf if compute == "bf16" else np.float32

    featp = np.zeros((NCORES, S * P, C), dtype=feat_np_dt)
    maskp = np.zeros((NCORES, S, NT, 128), dtype=np.float32)
    for b in range(B):
        c, s = divmod(b, S)
        n = counts[b]
        featp[c, s * P : s * P + n] = feat[offs[b] : offs[b + 1]].astype(feat_np_dt)
        maskp[c, s].reshape(-1)[:n] = 1.0
    # masks laid out [128, S*NT] so the DMA is contiguous
    maskd = maskp.transpose(0, 3, 1, 2).reshape(NCORES, 128, S * NT)

    # query-side fold (host; tiny)
    q = query[0]
    mu = q.mean(-1, keepdims=True)
    var = ((q - mu) ** 2).mean(-1, keepdims=True)
    qn = (q - mu) / np.sqrt(var + 1e-5) * g_q + b_q
    qh = (qn @ w_q.T + b_q_in) / np.sqrt(DH)  # [T, C]
    A = np.einsum(
        "thd,hdc->cht", qh.reshape(T, H, DH), w_k.reshape(H, DH, C)
    ).reshape(C, H * T)

    consts = dict(
        akT=np.ascontiguousarray(A.astype(feat_np_dt)),
        wvT=np.ascontiguousarray(w_v.T.astype(bf)),
        woT=np.ascontiguousarray(w_o.T.astype(bf)),
        w1gT=np.ascontiguousarray((w1 * g_ff[None, :]).T.astype(bf)),
        b1e=(b1 + w1 @ b_ff).reshape(1, 2 * C).astype(bf),
        w2T=np.ascontiguousarray(w2.T.astype(bf)),
        b2e=b2.reshape(1, C).astype(bf),
        qb=np.ascontiguousarray(query[0] + (b_o + w_o @ b_v_in)[None, :]).astype(
            np.float32
        ),
        ident=np.eye(128, dtype=feat_np_dt),
        identb=np.eye(128, dtype=bf),
    )
    in_maps = []
    for c in range(NCORES):
        m = dict(consts)
        m["featp"] = featp[c]
        m["maskp"] = maskd[c].astype(feat_np_dt)
        in_maps.append(m)
    return in_maps, P, S, B




COMPUTE = "bf16"


def kernel(**inputs):
    compute = COMPUTE
    from concourse.bass_utils import run_bass_kernel_spmd

    in_maps, P, S, B = _host_prep(inputs, compute)
    chunk = _pick_chunk(P // 128)
    key = (P, S, compute, chunk)
    if key not in _CACHE:
        _CACHE[key] = _build(P, S, compute=compute, chunk=chunk)
    nc = _CACHE[key]
    res = run_bass_kernel_spmd(nc, in_maps, core_ids=list(range(NCORES)))
    out = np.empty((B, T, C), dtype=np.float32)
    for c in range(NCORES):
        o = res.results[c]["outp"]
        for s in range(S):
            out[c * S + s] = o[s * T : (s + 1) * T]
    return out

